# revision 60
# baseline (speedup 1.0000x reference)
"""Multi-head attention (B=2, T=2048, C=1024, H=16) on 8 trn2 cores.

Sharding: core c -> batch b = c//4, head-group g = c%4 (4 heads, proj cols
[g*256, (g+1)*256)).  Host pre-transposes per-batch inputs to feature-major
[C, T] so every device matmul has its contraction dim on SBUF partitions.
Each core computes a transposed partial output  ypT = Wo_g^T @ O_g^T
[1024, 2048] (bf16); the host sums ypT.T over the 4 groups per batch and
adds bo.

Perf structure:
 - PE warm-up matmuls during the DMA-bound head (HAM unthrottle early).
 - Scores QK^T: two heads packed as 64-row PE tiles (auto tile_position).
 - PV: fp8e4 DoubleRow, two k-tiles per pass (contraction 256), with the
   ones-column trick for the softmax denominator (M=65).
 - Out-proj: transposed (ypT), head pairs packed on 128 partitions (K=128).
"""

import ml_dtypes
import numpy as np


import concourse.bass as bass
import concourse.tile as tile
from concourse import bacc, mybir
from concourse.bass_utils import run_bass_kernel_spmd

B, T, C, H, D = 2, 2048, 1024, 16, 64
N_CORES = 8
GROUPS = 4          # head-groups (cores per batch)
HG = H // GROUPS    # heads per core = 4
CG = HG * D         # proj cols per core = 256
KT = C // 128       # contraction k-tiles = 8
SCALE = D ** -0.5   # 1/8
import math
FEXP_A = SCALE * math.log2(math.e) * 128.0     # Schraudolph scale (bf16 bits)
FEXP_B = 127.0 * 128.0 - 7.41                  # Schraudolph shift, centered err
FEXP_TKS = (5, 11)      # k-tiles whose exp runs on DVE (fast-exp)
PV_LAG = 3              # PV consumes e_t this many k-tiles late

F32 = mybir.dt.float32
F32R = mybir.dt.float32r
BF16 = mybir.dt.bfloat16
I16 = mybir.dt.int16
AF = mybir.ActivationFunctionType
ALU = mybir.AluOpType


def build_mha_program():
    """Build the SPMD Bass program (identical on all 8 cores)."""
    nc = bacc.Bacc("TRN2", target_bir_lowering=False, debug=False,
                   num_devices=N_CORES)

    xqT = nc.dram_tensor("xqT", (C, T), BF16, kind="ExternalInput").ap()
    xkT = nc.dram_tensor("xkT", (C, T), BF16, kind="ExternalInput").ap()
    xvT = nc.dram_tensor("xvT", (C, T), BF16, kind="ExternalInput").ap()
    wq = nc.dram_tensor("wq", (C, CG), BF16, kind="ExternalInput").ap()
    wk = nc.dram_tensor("wk", (C, CG), BF16, kind="ExternalInput").ap()
    wv = nc.dram_tensor("wv", (C, CG), BF16, kind="ExternalInput").ap()
    bq = nc.dram_tensor("bq", (CG,), F32, kind="ExternalInput").ap()
    bk = nc.dram_tensor("bk", (CG,), F32, kind="ExternalInput").ap()
    bv = nc.dram_tensor("bv", (CG,), F32, kind="ExternalInput").ap()
    wo2 = nc.dram_tensor("wo2", (128, 2, C), BF16, kind="ExternalInput").ap()
    ypT = nc.dram_tensor("ypT", (C, T), BF16, kind="ExternalOutput").ap()
    # DRAM bounce buffer for the per-(qc,hp) reciprocal row broadcast
    rscr = nc.dram_tensor("rscr", (8, 2, 512), F32, kind="Internal").ap()

    with tile.TileContext(nc) as tc:
        _emit(tc, xqT, xkT, xvT, wq, wk, wv, bq, bk, bv, wo2, ypT, rscr)
    nc.compile()
    return nc


def _emit(tc, xqT, xkT, xvT, wq, wk, wv, bq, bk, bv, wo2, ypT, rscr):
    nc = tc.nc
    MT = CG // 128            # stationary tiles per projection = 2
    MC = 512                  # chunk width (tokens) everywhere
    NMC = T // MC             # 4 chunks
    TT = T // 128             # 16 t-tiles
    QC = 512                  # q-chunk width in attention
    NQC = T // QC             # 4 q-chunks
    VS = D + 1                # 65: V cols + ones col per head

    from contextlib import ExitStack
    with ExitStack() as ctx:
        consts = ctx.enter_context(tc.tile_pool(name="consts", bufs=1))
        xs_pool = ctx.enter_context(tc.tile_pool(name="xs", bufs=8))
        big = ctx.enter_context(tc.tile_pool(name="big", bufs=1))
        e_pool = ctx.enter_context(tc.tile_pool(name="e", bufs=6))
        ev_pool = ctx.enter_context(tc.tile_pool(name="ev", bufs=3))
        nrm_pool = ctx.enter_context(tc.tile_pool(name="nrm", bufs=4))
        # One shared PSUM ring (tag "S", 2x [128,2,512] = 4 banks) serves
        # scores, projections, out-proj and the norm broadcast; pv pool holds
        # the two in-flight PV accumulators (4 banks).  8 banks total.
        sp = ctx.enter_context(tc.tile_pool(name="sp", bufs=2, space="PSUM"))
        pv_ps = ctx.enter_context(tc.tile_pool(name="pvps", bufs=2, space="PSUM"))

        def stile(name):
            return sp.tile([128, 2, 512], F32, tag="S", name=name)

        # Per-chunk persistent activations.
        qTc = [big.tile([128, MT, MC], BF16, name=f"qTc{i}", tag=f"qTc{i}")
               for i in range(NMC)]
        kTc = [big.tile([128, MT, MC], BF16, name=f"kTc{i}", tag=f"kTc{i}")
               for i in range(NMC)]
        # V: [k-token part, head, sub-tile, VS]; col D is the ones column.
        vc = [big.tile([128, HG, MC // 128, VS], BF16, name=f"vc{i}",
                       tag=f"vc{i}") for i in range(NMC)]
        # Normalized attention out: head pair hp packed on 128 partitions
        # (even head on 0:64, odd head on 64:128).
        oc2 = [big.tile([128, 2, QC], BF16, name=f"oc{i}", tag=f"oc{i}")
               for i in range(NQC)]

        wq_s = consts.tile([128, KT, CG], BF16, tag="wq")
        wk_s = consts.tile([128, KT, CG], BF16, tag="wk")
        wv_s = consts.tile([128, KT, CG], BF16, tag="wv")
        wo_s = consts.tile([128, 2, C], BF16, tag="wo")
        bq_s = consts.tile([128, MT, 1], F32, tag="bq")
        bk_s = consts.tile([128, MT, 1], F32, tag="bk")
        bv_bc = consts.tile([128, CG], F32, tag="bv")
        ones_f = consts.tile([128, D], F32, tag="onesf")
        ones_t = consts.tile([128, D], F32R, tag="ones")
        wconst = consts.tile([128, 512], BF16, tag="wconst")

        def load_x(src, name, eng=None):
            x_t = xs_pool.tile([128, KT, MC], BF16, tag="xs", name=name)
            (eng or nc.sync).dma_start(
                out=x_t, in_=src.rearrange("(kt p) m -> p kt m", p=128))
            return x_t

        def emit_a(mc):
            cols = bass.ts(mc, MC)
            # chunk-0 inputs ride the scalar HWDGE queue, in parallel with
            # the weight DMAs on the sync queue (ACT is idle in the head)
            eng = nc.scalar if mc == 0 else None
            xq_t = load_x(xqT[:, cols], f"xq{mc}", eng)
            xk_t = load_x(xkT[:, cols], f"xk{mc}", eng)
            for x_t, w_s, b_s, dstl in ((xq_t, wq_s, bq_s, qTc),
                                        (xk_t, wk_s, bk_s, kTc)):
                ps = stile(f"pa{mc}")
                for mt in range(MT):
                    for kt in range(KT):
                        nc.tensor.matmul(
                            ps[:, mt, :],
                            w_s[:, kt, bass.ts(mt, 128)],
                            x_t[:, kt, :],
                            start=(kt == 0), stop=(kt == KT - 1))
                    nc.vector.tensor_scalar_add(
                        dstl[mc][:, mt, :], ps[:, mt, :], b_s[:, mt, :])

        def emit_b(mc):
            cols = bass.ts(mc, MC)
            v4 = vc[mc]
            nc.vector.memset(v4[:, :, :, D:VS], 1.0)
            xv_t = load_x(xvT[:, cols], f"xv{mc}",
                          nc.scalar if mc == 0 else None)
            ps = stile(f"pb{mc}")
            for sub in range(MC // 128):
                reg = ps[:, sub // 2, bass.ts(sub % 2, CG)]
                for kt in range(KT):
                    nc.tensor.matmul(
                        reg,
                        xv_t[:, kt, bass.ts(sub, 128)],
                        wv_s[:, kt, :],
                        start=(kt == 0), stop=(kt == KT - 1))
                nc.vector.tensor_add(
                    v4[:, :, sub, 0:D],
                    reg.rearrange("p (h c) -> p h c", h=HG),
                    bv_bc.rearrange("p (h c) -> p h c", h=HG))

        def emit_d_unit(qc, cp, tail=False):
            """ypT[cp*256:(cp+1)*256, qc*512:(qc+1)*512] — one S tile."""
            ps = stile(f"pd{qc}_{cp}")
            for i in range(2):
                for hp in range(2):
                    nc.tensor.matmul(
                        ps[:, i, :],
                        wo_s[:, hp, bass.ts(2 * cp + i, 128)],
                        oc2[qc][:, hp, :],
                        start=(hp == 0), stop=(hp == 1))
            ev = ev_pool.tile([128, 2, 512], BF16, tag="ev")
            if tail and cp % 2 == 1:      # split tail casts across engines
                nc.scalar.copy(ev, ps)
            else:
                nc.vector.tensor_copy(ev, ps)
            nc.sync.dma_start(
                out=ypT[bass.ds(cp * 256, 256), bass.ts(qc, QC)]
                .rearrange("(a p) t -> p a t", p=128),
                in_=ev)

        def c_iter(qc, hp):
            """Attention for head pair hp on q-chunk qc.  Yields after every
            tk so two head-pair chains interleave (keeps ACT saturated)."""
            po = pv_ps.tile([128, 2, 512], F32, tag="pv",
                            name=f"po{qc}_{hp}")

            def emit_pv(e_prev, tkp):
                for h01 in range(2):
                    nc.tensor.matmul(
                        po[0:VS, h01, :],
                        vc[tkp // 4][:, 2 * hp + h01, tkp % 4, :],
                        e_prev[:, h01, :],
                        start=(tkp == 0), stop=(tkp == TT - 1))

            pending = []      # PV lags exp by PV_LAG tks so a queued DVE/ACT
            for tk in range(TT):   # op can't stall the PE queue head
                ps = stile(f"sc{qc}_{hp}_{tk}")
                for h01 in range(2):
                    pb = h01 * D
                    nc.tensor.matmul(
                        ps[:, h01, :],
                        kTc[tk // 4][pb:pb + D, hp, bass.ts(tk % 4, 128)],
                        qTc[qc][pb:pb + D, hp, :],
                        start=True, stop=True)
                e_t = e_pool.tile([128, 2, 512], BF16, tag="e")
                if hp == 1 and tk % 4 != 0:
                    # Schraudolph fast-exp on DVE: bf16 bit pattern of
                    # 2^(s*SCALE*log2e) via int16 affine + convert.
                    nc.vector.tensor_scalar(
                        e_t.bitcast(I16), ps, FEXP_A, FEXP_B,
                        ALU.mult, ALU.add)
                else:
                    nc.scalar.activation(e_t, ps, AF.Exp, scale=SCALE)
                pending.append((e_t, tk))
                if len(pending) > PV_LAG:
                    emit_pv(*pending.pop(0))
                yield
            for u in pending:
                emit_pv(*u)
            yield
            den = nrm_pool.tile([128, 2, 512], F32R, tag="den")
            nc.scalar.copy(den[D:D + 1, :, :], po[D:D + 1, :, :])
            rb = stile(f"rb{qc}_{hp}")
            for h01 in range(2):
                nc.tensor.matmul(rb[0:D, h01, :],
                                 ones_t[D:D + 1, :],
                                 den[D:D + 1, h01, :],
                                 start=True, stop=True)
            rec = nrm_pool.tile([128, 2, 512], F32, tag="rec")
            nc.vector.reciprocal_approx_fast(rec[0:D, :, :], rb[0:D, :, :])
            for h01 in range(2):
                nc.vector.tensor_mul(
                    oc2[qc][64 * h01:64 * h01 + 64, hp, :],
                    po[0:D, h01, :], rec[0:D, h01, :])

        # ---- emission schedule (software pipeline) --------------------
        # PE warm-up: ~3.4us+ of dummy matmuls so HAM unthrottles early.
        # Kept short — with inputs split across two DMA queues the real
        # matmuls are ready by ~5us, and warm-up MMs block them in the
        # strict PE FIFO.
        nc.vector.memset(wconst, 0.0)
        for bank in range(1):
            wps = stile(f"warm{bank}")
            for i in range(16):
                nc.tensor.matmul(wps[:, bank % 2, :], wconst[:, 0:128], wconst,
                                 start=(i == 0), stop=(i == 15))
        nc.sync.dma_start(out=wq_s, in_=wq.rearrange("(kt p) c -> p kt c", p=128))
        nc.sync.dma_start(out=wk_s, in_=wk.rearrange("(kt p) c -> p kt c", p=128))
        nc.sync.dma_start(
            out=bq_s, in_=bq.rearrange("(mt p) -> p mt", p=128).unsqueeze(2))
        nc.sync.dma_start(
            out=bk_s, in_=bk.rearrange("(mt p) -> p mt", p=128).unsqueeze(2))
        nc.vector.memset(ones_f[D:D + 1, :], 1.0)
        # touch Exp once so the ACT table loads during the DMA-bound head
        warm = consts.tile([128, 1], F32, tag="warm")
        nc.vector.memset(warm[0:1, :], 0.0)
        nc.scalar.activation(warm[0:1, :], warm[0:1, :], AF.Exp)
        nc.vector.tensor_copy(ones_t[D:D + 1, :], ones_f[D:D + 1, :])

        emit_a(0)
        nc.sync.dma_start(out=wv_s, in_=wv.rearrange("(kt p) c -> p kt c", p=128))
        nc.sync.dma_start(
            out=bv_bc,
            in_=bass.AP(tensor=bv.tensor, offset=bv.offset,
                        ap=[[0, 128]] + list(bv.ap)))
        emit_b(0)
        nc.sync.dma_start(out=wo_s, in_=wo2)
        d_queue = []                         # (qc, cp) units, 4 MMs each
        for qc in range(NQC):
            genA = c_iter(qc, 0)
            genB = c_iter(qc, 1)
            for tk in range(TT):
                next(genA, None)
                next(genB, None)
                if qc == 0 and tk % 4 == 0 and tk // 4 + 1 < NMC:
                    emit_a(tk // 4 + 1)
                elif qc == 0 and tk % 4 == 1 and tk // 4 + 1 < NMC:
                    emit_b(tk // 4 + 1)
                if tk % 4 == 2 and d_queue:
                    emit_d_unit(*d_queue.pop(0))
            for _ in genA:                   # last PVs + norm (A then B)
                pass
            for _ in genB:
                pass
            d_queue += [(qc, cp) for cp in range(4)]
        for u in d_queue:
            emit_d_unit(*u, tail=True)


_NC_CACHE = None


def _get_program():
    global _NC_CACHE
    if _NC_CACHE is None:
        _NC_CACHE = build_mha_program()
    return _NC_CACHE


def make_in_maps(query, key, value, Wq, bq, Wk, bk, Wv, bv, Wo):
    q = np.asarray(query, np.float32).reshape(B, T, C)
    k = np.asarray(key, np.float32).reshape(B, T, C)
    v = np.asarray(value, np.float32).reshape(B, T, C)
    xT = {n: [np.ascontiguousarray(a[b].T).astype(ml_dtypes.bfloat16)
              for b in range(B)]
          for n, a in (("q", q), ("k", k), ("v", v))}
    in_maps = []
    for c in range(N_CORES):
        b, g = divmod(c, GROUPS)
        sl = slice(g * CG, (g + 1) * CG)
        wo_g = np.asarray(Wo, np.float32)[sl, :]          # [256, C]
        wo2 = np.ascontiguousarray(
            wo_g.reshape(2, 2, 64, C).transpose(1, 2, 0, 3).reshape(128, 2, C)
        ).astype(ml_dtypes.bfloat16)
        in_maps.append({
            "xqT": xT["q"][b], "xkT": xT["k"][b], "xvT": xT["v"][b],
            "wq": np.ascontiguousarray(np.asarray(Wq, np.float32)[:, sl]).astype(ml_dtypes.bfloat16),
            "wk": np.ascontiguousarray(np.asarray(Wk, np.float32)[:, sl]).astype(ml_dtypes.bfloat16),
            "wv": np.ascontiguousarray(np.asarray(Wv, np.float32)[:, sl]).astype(ml_dtypes.bfloat16),
            "bq": np.ascontiguousarray(np.asarray(bq, np.float32)[sl]),
            "bk": np.ascontiguousarray(np.asarray(bk, np.float32)[sl]),
            "bv": np.ascontiguousarray(np.asarray(bv, np.float32)[sl]),
            "wo2": wo2,
        })
    return in_maps


def assemble_output(results, bo):
    y = np.zeros((B, T, C), np.float32)
    for c, res in enumerate(results):
        y[c // GROUPS] += np.asarray(res["ypT"], np.float32).T
    y += np.asarray(bo, np.float32)
    return y


def kernel(query, key, value, Wq, bq, Wk, bk, Wv, bv, Wo, bo):
    nc = _get_program()
    in_maps = make_in_maps(query, key, value, Wq, bq, Wk, bk, Wv, bv, Wo)
    res = run_bass_kernel_spmd(nc, in_maps, list(range(N_CORES)))
    return assemble_output(res.results, bo)


# revision 62
# speedup vs baseline: 1.0214x; 1.0214x over previous
"""Multi-head attention (B=2, T=2048, C=1024, H=16) on 8 trn2 cores.

Sharding: core c -> batch b = c//4, head-group g = c%4 (4 heads, proj cols
[g*256, (g+1)*256)).  Host pre-transposes per-batch inputs to feature-major
[C, T] so every device matmul has its contraction dim on SBUF partitions.
Each core computes a transposed partial output  ypT = Wo_g^T @ O_g^T
[1024, 2048] (bf16); the host sums ypT.T over the 4 groups per batch and
adds bo.

Perf structure:
 - PE warm-up matmuls during the DMA-bound head (HAM unthrottle early).
 - Scores QK^T: two heads packed as 64-row PE tiles (auto tile_position).
 - PV: fp8e4 DoubleRow, two k-tiles per pass (contraction 256), with the
   ones-column trick for the softmax denominator (M=65).
 - Out-proj: transposed (ypT), head pairs packed on 128 partitions (K=128).
"""

import ml_dtypes
import numpy as np


import concourse.bass as bass
import concourse.tile as tile
from concourse import bacc, mybir
from concourse.bass_utils import run_bass_kernel_spmd

B, T, C, H, D = 2, 2048, 1024, 16, 64
N_CORES = 8
GROUPS = 4          # head-groups (cores per batch)
HG = H // GROUPS    # heads per core = 4
CG = HG * D         # proj cols per core = 256
KT = C // 128       # contraction k-tiles = 8
SCALE = D ** -0.5   # 1/8
import math
FEXP_A = SCALE * math.log2(math.e) * 128.0     # Schraudolph scale (bf16 bits)
FEXP_B = 127.0 * 128.0 - 7.41                  # Schraudolph shift, centered err
FEXP_TKS = (5, 11)      # k-tiles whose exp runs on DVE (fast-exp)
PV_LAG = 3              # PV consumes e_t this many k-tiles late

F32 = mybir.dt.float32
F32R = mybir.dt.float32r
BF16 = mybir.dt.bfloat16
I16 = mybir.dt.int16
AF = mybir.ActivationFunctionType
ALU = mybir.AluOpType


def build_mha_program():
    """Build the SPMD Bass program (identical on all 8 cores)."""
    nc = bacc.Bacc("TRN2", target_bir_lowering=False, debug=False,
                   num_devices=N_CORES)

    xqT = nc.dram_tensor("xqT", (C, T), BF16, kind="ExternalInput").ap()
    xkT = nc.dram_tensor("xkT", (C, T), BF16, kind="ExternalInput").ap()
    xvT = nc.dram_tensor("xvT", (C, T), BF16, kind="ExternalInput").ap()
    wq = nc.dram_tensor("wq", (C, CG), BF16, kind="ExternalInput").ap()
    wk = nc.dram_tensor("wk", (C, CG), BF16, kind="ExternalInput").ap()
    wv = nc.dram_tensor("wv", (C, CG), BF16, kind="ExternalInput").ap()
    bq = nc.dram_tensor("bq", (CG,), F32, kind="ExternalInput").ap()
    bk = nc.dram_tensor("bk", (CG,), F32, kind="ExternalInput").ap()
    bv = nc.dram_tensor("bv", (CG,), F32, kind="ExternalInput").ap()
    wo2 = nc.dram_tensor("wo2", (128, 2, C), BF16, kind="ExternalInput").ap()
    ypT = nc.dram_tensor("ypT", (C, T), BF16, kind="ExternalOutput").ap()
    # DRAM bounce buffer for the per-(qc,hp) reciprocal row broadcast
    rscr = nc.dram_tensor("rscr", (8, 2, 512), F32, kind="Internal").ap()

    with tile.TileContext(nc) as tc:
        _emit(tc, xqT, xkT, xvT, wq, wk, wv, bq, bk, bv, wo2, ypT, rscr)
    nc.compile()
    return nc


def _emit(tc, xqT, xkT, xvT, wq, wk, wv, bq, bk, bv, wo2, ypT, rscr):
    nc = tc.nc
    MT = CG // 128            # stationary tiles per projection = 2
    MC = 512                  # chunk width (tokens) everywhere
    NMC = T // MC             # 4 chunks
    TT = T // 128             # 16 t-tiles
    QC = 512                  # q-chunk width in attention
    NQC = T // QC             # 4 q-chunks
    VS = D + 1                # 65: V cols + ones col per head

    from contextlib import ExitStack
    with ExitStack() as ctx:
        consts = ctx.enter_context(tc.tile_pool(name="consts", bufs=1))
        xs_pool = ctx.enter_context(tc.tile_pool(name="xs", bufs=8))
        big = ctx.enter_context(tc.tile_pool(name="big", bufs=1))
        # 2 chains x (PV_LAG pending + 1 in flight) = 8 live e-tiles; 10
        # keeps the e-ring from coupling the chains' lookahead.
        e_pool = ctx.enter_context(tc.tile_pool(name="e", bufs=10))
        ev_pool = ctx.enter_context(tc.tile_pool(name="ev", bufs=3))
        nrm_pool = ctx.enter_context(tc.tile_pool(name="nrm", bufs=4))
        # One shared PSUM ring (tag "S", 2x [128,2,512] = 4 banks) serves
        # scores, projections, out-proj and the norm broadcast; pv pool holds
        # the two in-flight PV accumulators (4 banks).  8 banks total.
        sp = ctx.enter_context(tc.tile_pool(name="sp", bufs=2, space="PSUM"))
        pv_ps = ctx.enter_context(tc.tile_pool(name="pvps", bufs=2, space="PSUM"))

        def stile(name):
            return sp.tile([128, 2, 512], F32, tag="S", name=name)

        # Per-chunk persistent activations.
        qTc = [big.tile([128, MT, MC], BF16, name=f"qTc{i}", tag=f"qTc{i}")
               for i in range(NMC)]
        kTc = [big.tile([128, MT, MC], BF16, name=f"kTc{i}", tag=f"kTc{i}")
               for i in range(NMC)]
        # V: [k-token part, head, sub-tile, VS]; col D is the ones column.
        vc = [big.tile([128, HG, MC // 128, VS], BF16, name=f"vc{i}",
                       tag=f"vc{i}") for i in range(NMC)]
        # Normalized attention out: head pair hp packed on 128 partitions
        # (even head on 0:64, odd head on 64:128).
        oc2 = [big.tile([128, 2, QC], BF16, name=f"oc{i}", tag=f"oc{i}")
               for i in range(NQC)]

        wq_s = consts.tile([128, KT, CG], BF16, tag="wq")
        wk_s = consts.tile([128, KT, CG], BF16, tag="wk")
        wv_s = consts.tile([128, KT, CG], BF16, tag="wv")
        wo_s = consts.tile([128, 2, C], BF16, tag="wo")
        bq_s = consts.tile([128, MT, 1], F32, tag="bq")
        bk_s = consts.tile([128, MT, 1], F32, tag="bk")
        bv_bc = consts.tile([128, CG], F32, tag="bv")
        ones_f = consts.tile([128, D], F32, tag="onesf")
        ones_t = consts.tile([128, D], F32R, tag="ones")
        wconst = consts.tile([128, 512], BF16, tag="wconst")

        def load_x(src, name, eng=None):
            x_t = xs_pool.tile([128, KT, MC], BF16, tag="xs", name=name)
            (eng or nc.sync).dma_start(
                out=x_t, in_=src.rearrange("(kt p) m -> p kt m", p=128))
            return x_t

        def emit_a(mc):
            cols = bass.ts(mc, MC)
            # chunk-0 inputs ride the scalar HWDGE queue, in parallel with
            # the weight DMAs on the sync queue (ACT is idle in the head)
            eng = nc.scalar if mc == 0 else None
            xq_t = load_x(xqT[:, cols], f"xq{mc}", eng)
            xk_t = load_x(xkT[:, cols], f"xk{mc}", eng)
            for x_t, w_s, b_s, dstl in ((xq_t, wq_s, bq_s, qTc),
                                        (xk_t, wk_s, bk_s, kTc)):
                ps = stile(f"pa{mc}")
                for mt in range(MT):
                    for kt in range(KT):
                        nc.tensor.matmul(
                            ps[:, mt, :],
                            w_s[:, kt, bass.ts(mt, 128)],
                            x_t[:, kt, :],
                            start=(kt == 0), stop=(kt == KT - 1))
                    nc.vector.tensor_scalar_add(
                        dstl[mc][:, mt, :], ps[:, mt, :], b_s[:, mt, :])

        def emit_b(mc):
            cols = bass.ts(mc, MC)
            v4 = vc[mc]
            nc.vector.memset(v4[:, :, :, D:VS], 1.0)
            xv_t = load_x(xvT[:, cols], f"xv{mc}",
                          nc.scalar if mc == 0 else None)
            ps = stile(f"pb{mc}")
            for sub in range(MC // 128):
                reg = ps[:, sub // 2, bass.ts(sub % 2, CG)]
                for kt in range(KT):
                    nc.tensor.matmul(
                        reg,
                        xv_t[:, kt, bass.ts(sub, 128)],
                        wv_s[:, kt, :],
                        start=(kt == 0), stop=(kt == KT - 1))
                nc.vector.tensor_add(
                    v4[:, :, sub, 0:D],
                    reg.rearrange("p (h c) -> p h c", h=HG),
                    bv_bc.rearrange("p (h c) -> p h c", h=HG))

        def emit_d_unit(qc, cp, tail=False):
            """ypT[cp*256:(cp+1)*256, qc*512:(qc+1)*512] — one S tile."""
            ps = stile(f"pd{qc}_{cp}")
            for i in range(2):
                for hp in range(2):
                    nc.tensor.matmul(
                        ps[:, i, :],
                        wo_s[:, hp, bass.ts(2 * cp + i, 128)],
                        oc2[qc][:, hp, :],
                        start=(hp == 0), stop=(hp == 1))
            ev = ev_pool.tile([128, 2, 512], BF16, tag="ev")
            if tail and cp % 2 == 1:      # split tail casts across engines
                nc.scalar.copy(ev, ps)
            else:
                nc.vector.tensor_copy(ev, ps)
            nc.sync.dma_start(
                out=ypT[bass.ds(cp * 256, 256), bass.ts(qc, QC)]
                .rearrange("(a p) t -> p a t", p=128),
                in_=ev)

        def c_iter(qc, hp):
            """Attention for head pair hp on q-chunk qc.  Yields after every
            tk so two head-pair chains interleave (keeps ACT saturated)."""
            po = pv_ps.tile([128, 2, 512], F32, tag="pv",
                            name=f"po{qc}_{hp}")

            def emit_pv(e_prev, tkp):
                for h01 in range(2):
                    nc.tensor.matmul(
                        po[0:VS, h01, :],
                        vc[tkp // 4][:, 2 * hp + h01, tkp % 4, :],
                        e_prev[:, h01, :],
                        start=(tkp == 0), stop=(tkp == TT - 1))

            pending = []      # PV lags exp by PV_LAG tks so a queued DVE/ACT
            for tk in range(TT):   # op can't stall the PE queue head
                ps = stile(f"sc{qc}_{hp}_{tk}")
                for h01 in range(2):
                    pb = h01 * D
                    nc.tensor.matmul(
                        ps[:, h01, :],
                        kTc[tk // 4][pb:pb + D, hp, bass.ts(tk % 4, 128)],
                        qTc[qc][pb:pb + D, hp, :],
                        start=True, stop=True)
                e_t = e_pool.tile([128, 2, 512], BF16, tag="e")
                if hp == 1 and tk % 4 != 0:
                    # Schraudolph fast-exp on DVE: bf16 bit pattern of
                    # 2^(s*SCALE*log2e) via int16 affine + convert.
                    nc.vector.tensor_scalar(
                        e_t.bitcast(I16), ps, FEXP_A, FEXP_B,
                        ALU.mult, ALU.add)
                else:
                    nc.scalar.activation(e_t, ps, AF.Exp, scale=SCALE)
                pending.append((e_t, tk))
                if len(pending) > PV_LAG:
                    emit_pv(*pending.pop(0))
                yield
            for u in pending:
                emit_pv(*u)
            yield
            den = nrm_pool.tile([128, 2, 512], F32R, tag="den")
            nc.scalar.copy(den[D:D + 1, :, :], po[D:D + 1, :, :])
            rb = stile(f"rb{qc}_{hp}")
            for h01 in range(2):
                nc.tensor.matmul(rb[0:D, h01, :],
                                 ones_t[D:D + 1, :],
                                 den[D:D + 1, h01, :],
                                 start=True, stop=True)
            rec = nrm_pool.tile([128, 2, 512], F32, tag="rec")
            nc.vector.reciprocal_approx_fast(rec[0:D, :, :], rb[0:D, :, :])
            for h01 in range(2):
                nc.vector.tensor_mul(
                    oc2[qc][64 * h01:64 * h01 + 64, hp, :],
                    po[0:D, h01, :], rec[0:D, h01, :])

        # ---- emission schedule (software pipeline) --------------------
        # PE warm-up: dummy matmuls spanning the DMA-bound head (~12us) so
        # HAM unthrottles early and stays warm until real matmuls arrive.
        nc.vector.memset(wconst, 0.0)
        for bank in range(3):
            wps = stile(f"warm{bank}")
            for i in range(16):
                nc.tensor.matmul(wps[:, bank % 2, :], wconst[:, 0:128], wconst,
                                 start=(i == 0), stop=(i == 15))
        nc.sync.dma_start(out=wq_s, in_=wq.rearrange("(kt p) c -> p kt c", p=128))
        nc.sync.dma_start(out=wk_s, in_=wk.rearrange("(kt p) c -> p kt c", p=128))
        nc.sync.dma_start(
            out=bq_s, in_=bq.rearrange("(mt p) -> p mt", p=128).unsqueeze(2))
        nc.sync.dma_start(
            out=bk_s, in_=bk.rearrange("(mt p) -> p mt", p=128).unsqueeze(2))
        nc.vector.memset(ones_f[D:D + 1, :], 1.0)
        # touch Exp once so the ACT table loads during the DMA-bound head
        warm = consts.tile([128, 1], F32, tag="warm")
        nc.vector.memset(warm[0:1, :], 0.0)
        nc.scalar.activation(warm[0:1, :], warm[0:1, :], AF.Exp)
        nc.vector.tensor_copy(ones_t[D:D + 1, :], ones_f[D:D + 1, :])

        emit_a(0)
        nc.sync.dma_start(out=wv_s, in_=wv.rearrange("(kt p) c -> p kt c", p=128))
        nc.sync.dma_start(
            out=bv_bc,
            in_=bass.AP(tensor=bv.tensor, offset=bv.offset,
                        ap=[[0, 128]] + list(bv.ap)))
        emit_b(0)
        nc.sync.dma_start(out=wo_s, in_=wo2)
        d_queue = []                         # (qc, cp) units, 4 MMs each
        for qc in range(NQC):
            genA = c_iter(qc, 0)
            genB = c_iter(qc, 1)
            for tk in range(TT):
                next(genA, None)
                next(genB, None)
                if qc == 0 and tk % 4 == 0 and tk // 4 + 1 < NMC:
                    emit_a(tk // 4 + 1)
                elif qc == 0 and tk % 4 == 1 and tk // 4 + 1 < NMC:
                    emit_b(tk // 4 + 1)
                if tk % 4 == 2 and d_queue:
                    emit_d_unit(*d_queue.pop(0))
            for _ in genA:                   # last PVs + norm (A then B)
                pass
            for _ in genB:
                pass
            d_queue += [(qc, cp) for cp in range(4)]
        for u in d_queue:
            emit_d_unit(*u, tail=True)


_NC_CACHE = None


def _get_program():
    global _NC_CACHE
    if _NC_CACHE is None:
        _NC_CACHE = build_mha_program()
    return _NC_CACHE


def make_in_maps(query, key, value, Wq, bq, Wk, bk, Wv, bv, Wo):
    q = np.asarray(query, np.float32).reshape(B, T, C)
    k = np.asarray(key, np.float32).reshape(B, T, C)
    v = np.asarray(value, np.float32).reshape(B, T, C)
    xT = {n: [np.ascontiguousarray(a[b].T).astype(ml_dtypes.bfloat16)
              for b in range(B)]
          for n, a in (("q", q), ("k", k), ("v", v))}
    in_maps = []
    for c in range(N_CORES):
        b, g = divmod(c, GROUPS)
        sl = slice(g * CG, (g + 1) * CG)
        wo_g = np.asarray(Wo, np.float32)[sl, :]          # [256, C]
        wo2 = np.ascontiguousarray(
            wo_g.reshape(2, 2, 64, C).transpose(1, 2, 0, 3).reshape(128, 2, C)
        ).astype(ml_dtypes.bfloat16)
        in_maps.append({
            "xqT": xT["q"][b], "xkT": xT["k"][b], "xvT": xT["v"][b],
            "wq": np.ascontiguousarray(np.asarray(Wq, np.float32)[:, sl]).astype(ml_dtypes.bfloat16),
            "wk": np.ascontiguousarray(np.asarray(Wk, np.float32)[:, sl]).astype(ml_dtypes.bfloat16),
            "wv": np.ascontiguousarray(np.asarray(Wv, np.float32)[:, sl]).astype(ml_dtypes.bfloat16),
            "bq": np.ascontiguousarray(np.asarray(bq, np.float32)[sl]),
            "bk": np.ascontiguousarray(np.asarray(bk, np.float32)[sl]),
            "bv": np.ascontiguousarray(np.asarray(bv, np.float32)[sl]),
            "wo2": wo2,
        })
    return in_maps


def assemble_output(results, bo):
    y = np.zeros((B, T, C), np.float32)
    for c, res in enumerate(results):
        y[c // GROUPS] += np.asarray(res["ypT"], np.float32).T
    y += np.asarray(bo, np.float32)
    return y


def kernel(query, key, value, Wq, bq, Wk, bk, Wv, bv, Wo, bo):
    nc = _get_program()
    in_maps = make_in_maps(query, key, value, Wq, bq, Wk, bk, Wv, bv, Wo)
    res = run_bass_kernel_spmd(nc, in_maps, list(range(N_CORES)))
    return assemble_output(res.results, bo)
